# revision 41
# baseline (speedup 1.0000x reference)
"""Trainium2 Bass kernel for nn_Attention_Layer (dense transformer attention).

Computes, for X [N, D], Wq/Wk/Wv [D, D]:
    Q = X @ Wq.T ; K = X @ Wk.T ; V = X @ Wv.T
    O = softmax(Q @ K.T, axis=-1) @ V

Strategy (8 NeuronCores, SPMD single launch):
  - Shard rows of X across cores (N=8192 -> 1024 rows/core).
  - Each core computes K_b^T and V_b (each written to an internal DRAM
    bounce and all-gathered immediately, K first), then Q_b^T (kept in SBUF,
    its PE work hiding both collectives' latency).
  - Attention runs in the "transposed" layout: S^T[k, q] tiles are computed
    with K^T chunks stationary and Q^T moving; softmax uses a constant bias
    shift (exact after normalization; no per-row max needed since logits are
    bounded well inside fp32 exp range), so no on-chip transposes and no
    partition-axis reductions are ever needed.  P~ = exp(S^T + bias) chunks
    feed P@V directly as stationary operands; row-sums come from tiny N=2
    matmuls against a ones pair into a shared PSUM bank.  O accumulates in
    SBUF and is normalized once at the end.
  - All matmuls run as float32r (full PE rate at free-dim >= 256).

Overlap engineering (vs the v1 kernel):
  - Projections accumulate with the contraction chunk OUTERMOST across a
    wave of PSUM groups, so the first matmuls issue as soon as the first
    weight/input chunks land instead of after the full 8MB fill.
  - Wq streams into the SBUF buffers vacated by Wk (tag rotation), freeing
    32KB/partition so block 0's K^T/V stream tiles can prefetch during the
    Q projection; the exp activation table is preloaded at kernel start.
  - K^T bounce/gather tensors use a block-contiguous layout ([BPR*d, kb]
    per rank) so every stage-B K^T tile load is a handful of large
    descriptors instead of 128 strided ones; V stream loads ride the
    gpsimd DMA queue to keep the sync sequencer under control.
  - The final key block normalizes and stores each 128-query subtile as
    soon as its PV accumulation stops (row-sums for the block are finished
    first), hiding the normalize/writeout tail under the remaining PV
    matmuls.

AllGather concatenates rank blocks on axis 0; keys are processed in rank-block
order on every core, and the same (rank, local-row) indexing is used for both
K^T and V, so the softmax/PV reduction is consistent (softmax is permutation
invariant over keys).
"""

import numpy as np

import concourse.tile as tile
from concourse import bacc, mybir
from concourse.bass_utils import run_bass_kernel_spmd

N_CORES = 8
N_TOTAL = 8192
D_MODEL = 1024
R_PER_CORE = N_TOTAL // N_CORES  # 1024

F32 = mybir.dt.float32
EXP_BIAS = -45.0  # constant softmax shift; cancels exactly after normalization


def _mm_dt(use_f32r):
    return mybir.dt.float32r if use_f32r else mybir.dt.float32


def build_fused(
    n_cores=N_CORES,
    d=D_MODEL,
    r=R_PER_CORE,
    kb=512,
    exp_bias=EXP_BIAS,
    use_f32r=True,
    mock_ag=False,    # timing/sim builds: skip the collective, read own kvb
    repeat_attn=1,    # timing builds: run stage B this many times
    stream_bufs=2,    # double/triple buffering of the streamed K^T/V tiles
    ps_a_bufs=8,      # stage-A psum wave width (groups in flight)
    pt_bufs=2,        # P~ tile double-buffering across key blocks
    prefetch_blk0=True,  # load block 0's K^T/V during the Q projection
    dc_outer=False,   # contraction-outermost projection waves
    v_on_sync=True,   # stream V tiles on the sync queue (vs gpsimd SWDGE)
):
    """Build the fused QKV + AllGather + attention kernel (SPMD, one program).

    Per-core I/O:
      xt  [d, r]  ExternalInput  — X^T columns for this core's rows
      wqt/wkt/wvt [d, d] ExternalInput — W.T (replicated)
      o   [r, d]  ExternalOutput — this core's output rows
    """
    assert d % 128 == 0 and r % 128 == 0 and kb % 128 == 0
    DC = d // 128            # contraction chunks over d
    NQS = r // 128           # 128-query subtiles per core
    QG = min(512, r)         # query group (free dim) for S^T matmuls
    NQG = r // QG
    KC = kb // 128           # key chunks per key block
    BPR = r // kb            # key blocks per rank block
    DW = min(512, d)         # free-dim slice width over d
    ND = d // DW             # slices of d (for PV matmuls)
    RW = min(512, r)         # free-dim slice width over r
    NR = r // RW

    MM = _mm_dt(use_f32r)  # dtype of all matmul operands (producers round)

    nc = bacc.Bacc("TRN2", target_bir_lowering=False, debug=False, num_devices=n_cores)

    xt = nc.dram_tensor("xt", [d, r], MM, kind="ExternalInput").ap()
    wqt = nc.dram_tensor("wqt", [d, d], MM, kind="ExternalInput").ap()
    wkt = nc.dram_tensor("wkt", [d, d], MM, kind="ExternalInput").ap()
    wvt = nc.dram_tensor("wvt", [d, d], MM, kind="ExternalInput").ap()
    o = nc.dram_tensor("o", [r, d], F32, kind="ExternalOutput").ap()

    # Internal DRAM bounces: K_b^T and V_b, and their all-gathers.  Two
    # separate collectives so attention (which needs K^T + Q^T first) can
    # start while V is still gathering.  ktb is stored block-contiguously:
    # half-block `h` of the rank occupies rows [h*d, (h+1)*d) with kb
    # columns, so stage-B K^T tile loads are large linear descriptors.
    ktb = nc.dram_tensor("ktb", [BPR * d, kb], MM).ap()
    vb = nc.dram_tensor("vb", [r, d], MM).ap()
    ktg = nc.dram_tensor("ktg", [n_cores * BPR * d, kb], MM, addr_space="Shared").ap()
    vg = nc.dram_tensor("vg", [n_cores * r, d], MM, addr_space="Shared").ap()

    with tile.TileContext(nc) as tc:
        with tc.tile_pool(name="persist", bufs=1) as pp:
            # --- persistent tiles ---
            qt_t = []
            for dc in range(DC):
                t = pp.tile([128, r], MM, name=f"qt{dc}", tag=f"qt{dc}")
                qt_t.append(t)
            oacc = []
            for qs in range(NQS):
                t = pp.tile([128, d], F32, name=f"oacc{qs}", tag=f"oacc{qs}")
                oacc.append(t)
            oacc_rs = pp.tile([128, 2 * NQS], F32, name="oacc_rs", tag="oacc_rs")
            # ones pair (fp32r matmuls need even free dims, so the row-sum
            # is computed twice into adjacent psum columns)
            ones_t = pp.tile([128, 2], MM, name="ones_t", tag="ones_t")
            bias_t = pp.tile([128, 1], F32, name="bias_t", tag="bias_t")
            ones_f32 = pp.tile([128, 2], F32, name="ones_f32", tag="ones_f32")
            warm_t = pp.tile([128, 2], F32, name="warm_t", tag="warm_t")
            # block-0 stream prefetch tiles (filled during stage A)
            kt0_t, v0_t = [], []
            if prefetch_blk0:
                for dc in range(DC):
                    kt0_t.append(pp.tile([128, kb], MM, name=f"kt0_{dc}",
                                         tag=f"kt0_{dc}"))
                for kc in range(KC):
                    v0_t.append(pp.tile([128, d], MM, name=f"v0_{kc}",
                                        tag=f"v0_{kc}"))

            # ---------------- Stage A: projections ----------------
            with (
                tc.tile_pool(name="stage_a", bufs=1) as pa,
                tc.tile_pool(name="ps_a", bufs=ps_a_bufs, space="PSUM") as ps_a,
                tc.tile_pool(name="outs_a", bufs=2) as pout_a,
            ):
                # Wk/Wv stay resident; Wq rotates into Wk's buffers (tag
                # reuse with bufs=2) once the K projection retires them.
                # Issue order: (wk, xt) pairs first so the K projection can
                # start as soon as chunk 0 lands; wq goes on the scalar
                # queue so its pool-rotation wait can't stall the sync
                # queue's ktb/vb bounce writes.
                xt_t = []
                wk_t, wv_t, wq_t = [], [], []
                def load_in(t, dram_rows, width, engs, e0=0):
                    n = len(engs)
                    st = width // n
                    for i in range(n):
                        engs[(e0 + i) % n].dma_start(
                            out=t[:, i * st:(i + 1) * st],
                            in_=dram_rows[:, i * st:(i + 1) * st])

                # (wk, xt) fill: all column-first-halves before any second
                # half — K-proj groups whose operands live entirely in
                # columns 0:512 then gate on 4MB instead of the full 8MB
                h2 = d // 2
                for dc in range(DC):
                    t = pa.tile([128, d], MM, name=f"wk{dc}", tag=f"w{dc}", bufs=2)
                    (nc.sync if dc % 2 else nc.gpsimd).dma_start(
                        out=t[:, 0:h2], in_=wkt[dc * 128:(dc + 1) * 128, 0:h2])
                    wk_t.append(t)
                    t = pa.tile([128, r], MM, name=f"xt{dc}", tag=f"xt{dc}")
                    (nc.scalar if dc % 2 else nc.sync).dma_start(
                        out=t[:, 0:h2], in_=xt[dc * 128:(dc + 1) * 128, 0:h2])
                    xt_t.append(t)
                for dc in range(DC):
                    (nc.scalar if dc % 2 else nc.sync).dma_start(
                        out=xt_t[dc][:, h2:d],
                        in_=xt[dc * 128:(dc + 1) * 128, h2:d])
                    (nc.sync if dc % 2 else nc.gpsimd).dma_start(
                        out=wk_t[dc][:, h2:d],
                        in_=wkt[dc * 128:(dc + 1) * 128, h2:d])
                for dc in range(DC):
                    t = pa.tile([128, d], MM, name=f"wv{dc}", tag=f"w{dc}", bufs=2)
                    load_in(t, wvt[dc * 128:(dc + 1) * 128, :], d,
                            [nc.sync, nc.gpsimd], e0=dc)
                    wv_t.append(t)

                # constants + Exp act-table preload: issued after the input
                # loads so their engine/queue time never delays the first
                # DMAs; they only need to be ready for stage B
                nc.vector.memset(bias_t, exp_bias)
                nc.vector.memset(ones_f32, 1.0)
                nc.vector.tensor_copy(ones_t, ones_f32)
                nc.scalar.activation(
                    warm_t, ones_f32, mybir.ActivationFunctionType.Exp,
                    bias=bias_t, scale=1.0,
                )
                # wq rides the gpsimd queue: its loads BLOCK their queue
                # until the K projection retires wk's buffers (in-order
                # FIFO + pool rotation), and gpsimd has nothing behind them
                for dc in range(DC):
                    t = pa.tile([128, d], MM, name=f"wq{dc}", tag=f"w{dc}", bufs=2)
                    load_in(t, wqt[dc * 128:(dc + 1) * 128, :], d,
                            [nc.gpsimd, nc.gpsimd])
                    wq_t.append(t)

                def proj(groups, lhsT_of, rhs_of, sink, outer_waves=0,
                         wave=ps_a_bufs):
                    """out[g] = sum_dc lhsT_of(dc, g).T @ rhs_of(dc, g).

                    The first `outer_waves` waves run with the contraction
                    chunk OUTERMOST across the wave's PSUM groups, so the PE
                    has a full wave of matmuls runnable per arriving input
                    chunk (saturates the DMA-fill window); the rest run
                    group-at-a-time (better p-state behaviour once inputs
                    are resident).
                    """
                    done = 0
                    for _ in range(outer_waves):
                        wv_groups = groups[done:done + wave]
                        done += len(wv_groups)
                        ps_tiles = [
                            ps_a.tile([128, RW], F32, name="ps", tag="ps")
                            for _ in wv_groups
                        ]
                        for dc in range(DC):
                            for t, g in zip(ps_tiles, wv_groups):
                                nc.tensor.matmul(
                                    t, lhsT_of(dc, g), rhs_of(dc, g),
                                    start=(dc == 0), stop=(dc == DC - 1),
                                    skip_group_check=True,
                                )
                        for t, g in zip(ps_tiles, wv_groups):
                            sink(g, t)
                    for g in groups[done:]:
                        ps = ps_a.tile([128, RW], F32, name="ps", tag="ps")
                        for dc in range(DC):
                            nc.tensor.matmul(
                                ps, lhsT_of(dc, g), rhs_of(dc, g),
                                start=(dc == 0), stop=(dc == DC - 1),
                            )
                        sink(g, ps)

                # PSUM->SBUF sink copies alternate DVE / Activation so the
                # copy backlog never stalls PSUM buffer reuse
                _cp = [0]
                def sink_copy(dst, src):
                    if _cp[0] % 2 == 0:
                        nc.vector.tensor_copy(dst, src)
                    else:
                        nc.scalar.copy(dst, src)
                    _cp[0] += 1

                # K_b^T -> ktb (block-contiguous), then gather immediately
                def k_sink(g, ps):
                    oc, rg = g
                    ot = pout_a.tile([128, RW], MM, name="ot", tag="ot")
                    sink_copy(ot, ps)
                    # query... key columns rg*RW.. of K^T live in half-block
                    # rg (RW == kb) at rows oc*128..
                    for h in range(RW // kb):
                        hh = rg * (RW // kb) + h
                        nc.sync.dma_start(
                            out=ktb[hh * d + oc * 128:hh * d + (oc + 1) * 128, :],
                            in_=ot[:, h * kb:(h + 1) * kb],
                        )
                # group order follows the load gates: h0-only groups first
                NO2 = d // 256
                k_groups = (
                    [(oc, 0) for oc in range(NO2)]
                    + [(oc, 1) for oc in range(NO2)]
                    + [(oc, rg) for oc in range(NO2, d // 128)
                       for rg in range(NR)]
                )
                if NR == 1:
                    k_groups = [(oc, 0) for oc in range(d // 128)]
                proj(
                    k_groups,
                    lambda dc, g: wk_t[dc][:, g[0] * 128:(g[0] + 1) * 128],
                    lambda dc, g: xt_t[dc][:, g[1] * RW:(g[1] + 1) * RW],
                    k_sink,
                    outer_waves=1 if dc_outer else 0,
                )
                if not mock_ag:
                    nc.gpsimd.collective_compute(
                        "AllGather",
                        mybir.AluOpType.bypass,
                        ins=[ktb],
                        outs=[ktg],
                        replica_groups=[list(range(n_cores))],
                    )

                # V_b (natural layout) -> vb, then gather immediately so both
                # collectives are in flight while the Q projection runs; PV of
                # the first attention blocks then never waits on the gather.
                def v_sink(g, ps):
                    rc, og = g
                    ot = pout_a.tile([128, DW], MM, name="ot", tag="ot2")
                    sink_copy(ot, ps)
                    nc.sync.dma_start(
                        out=vb[rc * 128:(rc + 1) * 128, og * DW:(og + 1) * DW],
                        in_=ot,
                    )
                proj(
                    [(rc, og) for rc in range(r // 128) for og in range(ND)],
                    lambda dc, g: xt_t[dc][:, g[0] * 128:(g[0] + 1) * 128],
                    lambda dc, g: wv_t[dc][:, g[1] * DW:(g[1] + 1) * DW],
                    v_sink,
                )
                if not mock_ag:
                    nc.gpsimd.collective_compute(
                        "AllGather",
                        mybir.AluOpType.bypass,
                        ins=[vb],
                        outs=[vg],
                        replica_groups=[list(range(n_cores))],
                    )

                # prefetch block 0's K^T/V stream tiles; they only wait on
                # the gathers (or the local bounces under mock_ag), so the
                # DMAs fly while the PE is busy with the Q projection.
                kt_src, v_src = (ktb, vb) if mock_ag else (ktg, vg)
                if prefetch_blk0:
                    for dc in range(DC):
                        nc.sync.dma_start(
                            out=kt0_t[dc],
                            in_=kt_src[dc * 128:(dc + 1) * 128, :],
                        )
                    v_eng = nc.sync if v_on_sync else nc.gpsimd
                    for kc in range(KC):
                        v_eng.dma_start(
                            out=v0_t[kc],
                            in_=v_src[kc * 128:(kc + 1) * 128, :],
                        )

                # Q_b^T stays in SBUF (PE work that overlaps both gathers)
                def q_sink(g, ps):
                    oc, rg = g
                    sink_copy(qt_t[oc][:, rg * RW:(rg + 1) * RW], ps)
                proj(
                    [(oc, rg) for oc in range(d // 128) for rg in range(NR)],
                    lambda dc, g: wq_t[dc][:, g[0] * 128:(g[0] + 1) * 128],
                    lambda dc, g: xt_t[dc][:, g[1] * RW:(g[1] + 1) * RW],
                    q_sink,
                )

            # ---------------- Stage B: attention ----------------
            with (
                tc.tile_pool(name="stream", bufs=stream_bufs) as pstream,
                tc.tile_pool(name="pt_pool", bufs=pt_bufs) as ppt,
                tc.tile_pool(name="ps_st", bufs=3, space="PSUM") as ps_st,
                tc.tile_pool(name="ps_pv", bufs=4, space="PSUM") as ps_pv,
                tc.tile_pool(name="ps_rs", bufs=1, space="PSUM") as ps_rs,
                tc.tile_pool(name="outp", bufs=2) as pout,
            ):
                n_blocks = n_cores * BPR
                total_blocks = repeat_attn * n_blocks
                for blk_i in range(total_blocks):
                    blk = blk_i % n_blocks
                    rank = blk // BPR
                    half = blk % BPR
                    last = blk_i == total_blocks - 1
                    if mock_ag:
                        kt_row0 = half * d
                        v_row0 = half * kb
                    else:
                        kt_row0 = (rank * BPR + half) * d
                        v_row0 = rank * r + half * kb

                    if blk_i == 0 and prefetch_blk0:
                        kt_t, v_t = kt0_t, v0_t
                    else:
                        kt_t = []
                        for dc in range(DC):
                            t = pstream.tile([128, kb], MM, name=f"kt{dc}",
                                             tag=f"kt{dc}")
                            nc.sync.dma_start(
                                out=t,
                                in_=kt_src[kt_row0 + dc * 128:
                                           kt_row0 + (dc + 1) * 128, :],
                            )
                            kt_t.append(t)
                        v_t = []
                        v_eng = nc.sync if v_on_sync else nc.gpsimd
                        for kc in range(KC):
                            t = pstream.tile([128, d], MM, name=f"v{kc}",
                                             tag=f"v{kc}")
                            v_eng.dma_start(
                                out=t,
                                in_=v_src[v_row0 + kc * 128:
                                          v_row0 + (kc + 1) * 128, :],
                            )
                            v_t.append(t)

                    # S^T = K_chunk @ Q^T ; P~ = exp(S^T + bias)
                    pt_t = {}
                    for kc in range(KC):
                        for qg in range(NQG):
                            ps = ps_st.tile([128, QG], F32, name="st_ps", tag="st_ps")
                            for dc in range(DC):
                                nc.tensor.matmul(
                                    ps,
                                    kt_t[dc][:, kc * 128:(kc + 1) * 128],
                                    qt_t[dc][:, qg * QG:(qg + 1) * QG],
                                    start=(dc == 0),
                                    stop=(dc == DC - 1),
                                )
                            pt = ppt.tile([128, QG], MM, name="pt", tag=f"pt{kc}_{qg}")
                            nc.scalar.activation(
                                pt, ps, mybir.ActivationFunctionType.Exp,
                                bias=bias_t, scale=1.0,
                            )
                            pt_t[(kc, qg)] = pt

                    rs = ps_rs.tile([128, 2 * NQS], F32, name="rs_ps", tag="rs_ps")
                    if last:
                        # finish all row-sums first so every subtile can be
                        # normalized + stored the moment its PV stops
                        for qs in range(NQS):
                            qg, off = divmod(qs * 128, QG)
                            for kc in range(KC):
                                nc.tensor.matmul(
                                    rs[:, 2 * qs:2 * qs + 2],
                                    pt_t[(kc, qg)][:, off:off + 128],
                                    ones_t,
                                    start=(kc == 0),
                                    stop=(kc == KC - 1),
                                    skip_group_check=True,
                                )
                        nc.vector.tensor_add(oacc_rs, oacc_rs, rs)
                        recip = pout.tile([128, 2 * NQS], F32, name="recip",
                                          tag="recip", bufs=1)
                        nc.vector.reciprocal(recip, oacc_rs)

                    # O += P~^T.T @ V in half-d PSUM groups (nd outermost,
                    # same matmul count/cycles): each 512-wide half retires
                    # as soon as its kc accumulation stops, so the add (and,
                    # in the last block, normalize+store) pipelines behind
                    # the PE at half-subtile granularity — the exposed tail
                    # is only the final half's chain.  Row-sums ride the
                    # nd==0 pass (already done up front in the last block).
                    for qs in range(NQS):
                        qg, off = divmod(qs * 128, QG)
                        lhsTs = [pt_t[(kc, qg)][:, off:off + 128]
                                 for kc in range(KC)]
                        for nd in range(ND):
                            pvh = ps_pv.tile([128, DW], F32, name="pv_ps",
                                             tag="pv_ps")
                            for kc in range(KC):
                                nc.tensor.matmul(
                                    pvh,
                                    lhsTs[kc],
                                    v_t[kc][:, nd * DW:(nd + 1) * DW],
                                    start=(kc == 0),
                                    stop=(kc == KC - 1),
                                    skip_group_check=True,
                                )
                                if nd == 0 and not last:
                                    nc.tensor.matmul(
                                        rs[:, 2 * qs:2 * qs + 2],
                                        lhsTs[kc],
                                        ones_t,
                                        start=(kc == 0),
                                        stop=(kc == KC - 1),
                                        skip_group_check=True,
                                    )
                            sl = slice(nd * DW, (nd + 1) * DW)
                            if last:
                                # add on DVE, normalize on the (idle)
                                # Activation engine: each runs under the
                                # PE's 852ns/half cadence, where a serial
                                # DVE add+mul chain would backlog
                                nc.vector.tensor_add(
                                    oacc[qs][:, sl], oacc[qs][:, sl], pvh)
                                nc.scalar.mul(
                                    oacc[qs][:, sl], oacc[qs][:, sl],
                                    recip[:, 2 * qs:2 * qs + 1])
                                # early halves drain on the slow gpsimd
                                # queue; the final ones ride HWDGE
                                idx = 2 * qs + nd
                                if idx < 5:
                                    w_eng = nc.gpsimd
                                else:
                                    w_eng = nc.sync if idx % 2 else nc.scalar
                                w_eng.dma_start(
                                    out=o[qs * 128:(qs + 1) * 128, sl],
                                    in_=oacc[qs][:, sl])
                            elif blk_i == 0:
                                nc.vector.tensor_copy(oacc[qs][:, sl], pvh)
                            else:
                                nc.vector.tensor_add(
                                    oacc[qs][:, sl], oacc[qs][:, sl], pvh)
                    if not last:
                        if blk_i == 0:
                            nc.vector.tensor_copy(oacc_rs, rs)
                        else:
                            nc.vector.tensor_add(oacc_rs, oacc_rs, rs)

    nc.compile()
    return nc


_NC_CACHE = {}


def _get_nc():
    if "fused" not in _NC_CACHE:
        _NC_CACHE["fused"] = build_fused()
    return _NC_CACHE["fused"]


def kernel(inputs, Wq, Wk, Wv):
    inputs = np.ascontiguousarray(inputs, dtype=np.float32)
    XT = np.ascontiguousarray(inputs.T)
    WqT = np.ascontiguousarray(np.asarray(Wq, dtype=np.float32).T)
    WkT = np.ascontiguousarray(np.asarray(Wk, dtype=np.float32).T)
    WvT = np.ascontiguousarray(np.asarray(Wv, dtype=np.float32).T)

    nc = _get_nc()
    R = R_PER_CORE
    in_maps = [
        {
            "xt": np.ascontiguousarray(XT[:, c * R:(c + 1) * R]),
            "wqt": WqT,
            "wkt": WkT,
            "wvt": WvT,
        }
        for c in range(N_CORES)
    ]
    res = run_bass_kernel_spmd(nc, in_maps, core_ids=list(range(N_CORES)))
    out = np.concatenate([res.results[c]["o"] for c in range(N_CORES)], axis=0)
    return out.astype(np.float32)


# revision 45
# speedup vs baseline: 1.0559x; 1.0559x over previous
"""Trainium2 Bass kernel for nn_Attention_Layer (dense transformer attention).

Computes, for X [N, D], Wq/Wk/Wv [D, D]:
    Q = X @ Wq.T ; K = X @ Wk.T ; V = X @ Wv.T
    O = softmax(Q @ K.T, axis=-1) @ V

Strategy (8 NeuronCores, SPMD single launch):
  - Shard rows of X across cores (N=8192 -> 1024 rows/core).
  - Each core computes K_b^T and V_b (each written to an internal DRAM
    bounce and all-gathered immediately, K first), then Q_b^T (kept in SBUF,
    its PE work hiding both collectives' latency).
  - Attention runs in the "transposed" layout: S^T[k, q] tiles are computed
    with K^T chunks stationary and Q^T moving; softmax uses a constant bias
    shift (exact after normalization; no per-row max needed since logits are
    bounded well inside fp32 exp range), so no on-chip transposes and no
    partition-axis reductions are ever needed.  P~ = exp(S^T + bias) chunks
    feed P@V directly as stationary operands; row-sums come from tiny N=2
    matmuls against a ones pair into a shared PSUM bank.  O accumulates in
    SBUF and is normalized once at the end.
  - All matmuls run as float32r (full PE rate at free-dim >= 256).

Overlap engineering (vs the v1 kernel):
  - Projections accumulate with the contraction chunk OUTERMOST across a
    wave of PSUM groups, so the first matmuls issue as soon as the first
    weight/input chunks land instead of after the full 8MB fill.
  - Wq streams into the SBUF buffers vacated by Wk (tag rotation), freeing
    32KB/partition so block 0's K^T/V stream tiles can prefetch during the
    Q projection; the exp activation table is preloaded at kernel start.
  - K^T bounce/gather tensors use a block-contiguous layout ([BPR*d, kb]
    per rank) so every stage-B K^T tile load is a handful of large
    descriptors instead of 128 strided ones; V stream loads ride the
    gpsimd DMA queue to keep the sync sequencer under control.
  - The final key block normalizes and stores each 128-query subtile as
    soon as its PV accumulation stops (row-sums for the block are finished
    first), hiding the normalize/writeout tail under the remaining PV
    matmuls.

AllGather concatenates rank blocks on axis 0; keys are processed in rank-block
order on every core, and the same (rank, local-row) indexing is used for both
K^T and V, so the softmax/PV reduction is consistent (softmax is permutation
invariant over keys).
"""

import numpy as np

import concourse.tile as tile
from concourse import bacc, mybir
from concourse.bass_utils import run_bass_kernel_spmd

N_CORES = 8
N_TOTAL = 8192
D_MODEL = 1024
R_PER_CORE = N_TOTAL // N_CORES  # 1024

F32 = mybir.dt.float32
EXP_BIAS = -45.0  # constant softmax shift; cancels exactly after normalization


def _mm_dt(use_f32r):
    return mybir.dt.float32r if use_f32r else mybir.dt.float32


def build_fused(
    n_cores=N_CORES,
    d=D_MODEL,
    r=R_PER_CORE,
    kb=512,
    exp_bias=EXP_BIAS,
    use_f32r=True,
    mock_ag=False,    # timing/sim builds: skip the collective, read own kvb
    repeat_attn=1,    # timing builds: run stage B this many times
    stream_bufs=2,    # double/triple buffering of the streamed K^T/V tiles
    ps_a_bufs=8,      # stage-A psum wave width (groups in flight)
    pt_bufs=2,        # P~ tile double-buffering across key blocks
    prefetch_blk0=True,  # load block 0's K^T/V during the Q projection
    dc_outer=False,   # contraction-outermost projection waves
    v_on_sync=True,   # stream V tiles on the sync queue (vs gpsimd SWDGE)
):
    """Build the fused QKV + AllGather + attention kernel (SPMD, one program).

    Per-core I/O:
      xt  [d, r]  ExternalInput  — X^T columns for this core's rows
      wqt/wkt/wvt [d, d] ExternalInput — W.T (replicated)
      o   [r, d]  ExternalOutput — this core's output rows
    """
    assert d % 128 == 0 and r % 128 == 0 and kb % 128 == 0
    DC = d // 128            # contraction chunks over d
    NQS = r // 128           # 128-query subtiles per core
    QG = min(512, r)         # query group (free dim) for S^T matmuls
    NQG = r // QG
    KC = kb // 128           # key chunks per key block
    BPR = r // kb            # key blocks per rank block
    DW = min(512, d)         # free-dim slice width over d
    ND = d // DW             # slices of d (for PV matmuls)
    RW = min(512, r)         # free-dim slice width over r
    NR = r // RW

    MM = _mm_dt(use_f32r)  # dtype of all matmul operands (producers round)

    nc = bacc.Bacc("TRN2", target_bir_lowering=False, debug=False, num_devices=n_cores)

    xt = nc.dram_tensor("xt", [d, r], MM, kind="ExternalInput").ap()
    wqt = nc.dram_tensor("wqt", [d, d], MM, kind="ExternalInput").ap()
    wkt = nc.dram_tensor("wkt", [d, d], MM, kind="ExternalInput").ap()
    wvt = nc.dram_tensor("wvt", [d, d], MM, kind="ExternalInput").ap()
    o = nc.dram_tensor("o", [r, d], F32, kind="ExternalOutput").ap()

    # Internal DRAM bounces: K_b^T and V_b, and their all-gathers.  Two
    # separate collectives so attention (which needs K^T + Q^T first) can
    # start while V is still gathering.  ktb is stored block-contiguously:
    # half-block `h` of the rank occupies rows [h*d, (h+1)*d) with kb
    # columns, so stage-B K^T tile loads are large linear descriptors.
    ktb = nc.dram_tensor("ktb", [BPR * d, kb], MM).ap()
    vb = nc.dram_tensor("vb", [r, d], MM).ap()
    ktg = nc.dram_tensor("ktg", [n_cores * BPR * d, kb], MM, addr_space="Shared").ap()
    vg = nc.dram_tensor("vg", [n_cores * r, d], MM, addr_space="Shared").ap()

    with tile.TileContext(nc) as tc:
        with tc.tile_pool(name="persist", bufs=1) as pp:
            # --- persistent tiles ---
            qt_t = []
            for dc in range(DC):
                t = pp.tile([128, r], MM, name=f"qt{dc}", tag=f"qt{dc}")
                qt_t.append(t)
            oacc = []
            for qs in range(NQS):
                t = pp.tile([128, d], F32, name=f"oacc{qs}", tag=f"oacc{qs}")
                oacc.append(t)
            oacc_rs = pp.tile([128, 2 * NQS], F32, name="oacc_rs", tag="oacc_rs")
            # ones pair (fp32r matmuls need even free dims, so the row-sum
            # is computed twice into adjacent psum columns)
            ones_t = pp.tile([128, 2], MM, name="ones_t", tag="ones_t")
            bias_t = pp.tile([128, 1], F32, name="bias_t", tag="bias_t")
            ones_f32 = pp.tile([128, 2], F32, name="ones_f32", tag="ones_f32")
            warm_t = pp.tile([128, 2], F32, name="warm_t", tag="warm_t")
            # block-0 stream prefetch tiles (filled during stage A)
            kt0_t, v0_t = [], []
            if prefetch_blk0:
                for dc in range(DC):
                    kt0_t.append(pp.tile([128, kb], MM, name=f"kt0_{dc}",
                                         tag=f"kt0_{dc}"))
                for kc in range(KC):
                    v0_t.append(pp.tile([128, d], MM, name=f"v0_{kc}",
                                        tag=f"v0_{kc}"))

            # ---------------- Stage A: projections ----------------
            with (
                tc.tile_pool(name="stage_a", bufs=1) as pa,
                tc.tile_pool(name="ps_a", bufs=ps_a_bufs, space="PSUM") as ps_a,
                tc.tile_pool(name="outs_a", bufs=2) as pout_a,
            ):
                # Wk/Wv stay resident; Wq rotates into Wk's buffers (tag
                # reuse with bufs=2) once the K projection retires them.
                # Issue order: (wk, xt) pairs first so the K projection can
                # start as soon as chunk 0 lands; wq goes on the scalar
                # queue so its pool-rotation wait can't stall the sync
                # queue's ktb/vb bounce writes.
                xt_t = []
                wk_t, wv_t, wq_t = [], [], []
                def load_in(t, dram_rows, width, engs, e0=0):
                    n = len(engs)
                    st = width // n
                    for i in range(n):
                        engs[(e0 + i) % n].dma_start(
                            out=t[:, i * st:(i + 1) * st],
                            in_=dram_rows[:, i * st:(i + 1) * st])

                # (wk, xt) fill: all column-first-halves before any second
                # half — K-proj groups whose operands live entirely in
                # columns 0:512 then gate on 4MB instead of the full 8MB
                h2 = d // 2
                for dc in range(DC):
                    t = pa.tile([128, d], MM, name=f"wk{dc}", tag=f"w{dc}", bufs=2)
                    (nc.sync if dc % 2 else nc.gpsimd).dma_start(
                        out=t[:, 0:h2], in_=wkt[dc * 128:(dc + 1) * 128, 0:h2])
                    wk_t.append(t)
                    t = pa.tile([128, r], MM, name=f"xt{dc}", tag=f"xt{dc}")
                    (nc.scalar if dc % 2 else nc.sync).dma_start(
                        out=t[:, 0:h2], in_=xt[dc * 128:(dc + 1) * 128, 0:h2])
                    xt_t.append(t)
                for dc in range(DC):
                    (nc.scalar if dc % 2 else nc.sync).dma_start(
                        out=xt_t[dc][:, h2:d],
                        in_=xt[dc * 128:(dc + 1) * 128, h2:d])
                    (nc.sync if dc % 2 else nc.gpsimd).dma_start(
                        out=wk_t[dc][:, h2:d],
                        in_=wkt[dc * 128:(dc + 1) * 128, h2:d])
                for dc in range(DC):
                    t = pa.tile([128, d], MM, name=f"wv{dc}", tag=f"w{dc}", bufs=2)
                    load_in(t, wvt[dc * 128:(dc + 1) * 128, :], d,
                            [nc.sync, nc.gpsimd], e0=dc)
                    wv_t.append(t)

                # constants + Exp act-table preload: issued after the input
                # loads so their engine/queue time never delays the first
                # DMAs; they only need to be ready for stage B
                nc.vector.memset(bias_t, exp_bias)
                nc.vector.memset(ones_f32, 1.0)
                nc.vector.tensor_copy(ones_t, ones_f32)
                nc.scalar.activation(
                    warm_t, ones_f32, mybir.ActivationFunctionType.Exp,
                    bias=bias_t, scale=1.0,
                )
                # wq rides the gpsimd queue: its loads BLOCK their queue
                # until the K projection retires wk's buffers (in-order
                # FIFO + pool rotation), and gpsimd has nothing behind them
                for dc in range(DC):
                    t = pa.tile([128, d], MM, name=f"wq{dc}", tag=f"w{dc}", bufs=2)
                    load_in(t, wqt[dc * 128:(dc + 1) * 128, :], d,
                            [nc.gpsimd, nc.gpsimd])
                    wq_t.append(t)

                def proj(groups, lhsT_of, rhs_of, sink, outer_waves=0,
                         wave=ps_a_bufs):
                    """out[g] = sum_dc lhsT_of(dc, g).T @ rhs_of(dc, g).

                    The first `outer_waves` waves run with the contraction
                    chunk OUTERMOST across the wave's PSUM groups, so the PE
                    has a full wave of matmuls runnable per arriving input
                    chunk (saturates the DMA-fill window); the rest run
                    group-at-a-time (better p-state behaviour once inputs
                    are resident).
                    """
                    done = 0
                    for _ in range(outer_waves):
                        wv_groups = groups[done:done + wave]
                        done += len(wv_groups)
                        ps_tiles = [
                            ps_a.tile([128, RW], F32, name="ps", tag="ps")
                            for _ in wv_groups
                        ]
                        for dc in range(DC):
                            for t, g in zip(ps_tiles, wv_groups):
                                nc.tensor.matmul(
                                    t, lhsT_of(dc, g), rhs_of(dc, g),
                                    start=(dc == 0), stop=(dc == DC - 1),
                                    skip_group_check=True,
                                )
                        for t, g in zip(ps_tiles, wv_groups):
                            sink(g, t)
                    for g in groups[done:]:
                        ps = ps_a.tile([128, RW], F32, name="ps", tag="ps")
                        for dc in range(DC):
                            nc.tensor.matmul(
                                ps, lhsT_of(dc, g), rhs_of(dc, g),
                                start=(dc == 0), stop=(dc == DC - 1),
                            )
                        sink(g, ps)

                # PSUM->SBUF sink copies alternate DVE / Activation so the
                # copy backlog never stalls PSUM buffer reuse
                _cp = [0]
                def sink_copy(dst, src):
                    if _cp[0] % 2 == 0:
                        nc.vector.tensor_copy(dst, src)
                    else:
                        nc.scalar.copy(dst, src)
                    _cp[0] += 1

                # K_b^T -> ktb (block-contiguous), then gather immediately
                def k_sink(g, ps):
                    oc, rg = g
                    ot = pout_a.tile([128, RW], MM, name="ot", tag="ot")
                    sink_copy(ot, ps)
                    # query... key columns rg*RW.. of K^T live in half-block
                    # rg (RW == kb) at rows oc*128..
                    for h in range(RW // kb):
                        hh = rg * (RW // kb) + h
                        nc.sync.dma_start(
                            out=ktb[hh * d + oc * 128:hh * d + (oc + 1) * 128, :],
                            in_=ot[:, h * kb:(h + 1) * kb],
                        )
                # group order follows the load gates: h0-only groups first
                NO2 = d // 256
                k_groups = (
                    [(oc, 0) for oc in range(NO2)]
                    + [(oc, 1) for oc in range(NO2)]
                    + [(oc, rg) for oc in range(NO2, d // 128)
                       for rg in range(NR)]
                )
                if NR == 1:
                    k_groups = [(oc, 0) for oc in range(d // 128)]
                proj(
                    k_groups,
                    lambda dc, g: wk_t[dc][:, g[0] * 128:(g[0] + 1) * 128],
                    lambda dc, g: xt_t[dc][:, g[1] * RW:(g[1] + 1) * RW],
                    k_sink,
                    outer_waves=1 if dc_outer else 0,
                )
                if not mock_ag:
                    nc.gpsimd.collective_compute(
                        "AllGather",
                        mybir.AluOpType.bypass,
                        ins=[ktb],
                        outs=[ktg],
                        replica_groups=[list(range(n_cores))],
                    )

                # V_b (natural layout) -> vb, then gather immediately so both
                # collectives are in flight while the Q projection runs; PV of
                # the first attention blocks then never waits on the gather.
                def v_sink(g, ps):
                    rc, og = g
                    ot = pout_a.tile([128, DW], MM, name="ot", tag="ot2")
                    sink_copy(ot, ps)
                    nc.sync.dma_start(
                        out=vb[rc * 128:(rc + 1) * 128, og * DW:(og + 1) * DW],
                        in_=ot,
                    )
                proj(
                    [(rc, og) for rc in range(r // 128) for og in range(ND)],
                    lambda dc, g: xt_t[dc][:, g[0] * 128:(g[0] + 1) * 128],
                    lambda dc, g: wv_t[dc][:, g[1] * DW:(g[1] + 1) * DW],
                    v_sink,
                )
                if not mock_ag:
                    nc.gpsimd.collective_compute(
                        "AllGather",
                        mybir.AluOpType.bypass,
                        ins=[vb],
                        outs=[vg],
                        replica_groups=[list(range(n_cores))],
                    )

                # prefetch block 0's K^T/V stream tiles; they only wait on
                # the gathers (or the local bounces under mock_ag), so the
                # DMAs fly while the PE is busy with the Q projection.
                kt_src, v_src = (ktb, vb) if mock_ag else (ktg, vg)
                if prefetch_blk0:
                    for dc in range(DC):
                        nc.sync.dma_start(
                            out=kt0_t[dc],
                            in_=kt_src[dc * 128:(dc + 1) * 128, :],
                        )
                    v_eng = nc.sync if v_on_sync else nc.gpsimd
                    for kc in range(KC):
                        v_eng.dma_start(
                            out=v0_t[kc],
                            in_=v_src[kc * 128:(kc + 1) * 128, :],
                        )

                # Q_b^T stays in SBUF (PE work that overlaps both gathers)
                def q_sink(g, ps):
                    oc, rg = g
                    sink_copy(qt_t[oc][:, rg * RW:(rg + 1) * RW], ps)
                proj(
                    [(oc, rg) for oc in range(d // 128) for rg in range(NR)],
                    lambda dc, g: wq_t[dc][:, g[0] * 128:(g[0] + 1) * 128],
                    lambda dc, g: xt_t[dc][:, g[1] * RW:(g[1] + 1) * RW],
                    q_sink,
                )

            # ---------------- Stage B: attention ----------------
            with (
                tc.tile_pool(name="stream", bufs=stream_bufs) as pstream,
                tc.tile_pool(name="pt_pool", bufs=pt_bufs) as ppt,
                tc.tile_pool(name="ps_st", bufs=3, space="PSUM") as ps_st,
                tc.tile_pool(name="ps_pv", bufs=4, space="PSUM") as ps_pv,
                tc.tile_pool(name="ps_rs", bufs=1, space="PSUM") as ps_rs,
                tc.tile_pool(name="outp", bufs=2) as pout,
            ):
                n_blocks = n_cores * BPR
                total_blocks = repeat_attn * n_blocks
                for blk_i in range(total_blocks):
                    blk = blk_i % n_blocks
                    rank = blk // BPR
                    half = blk % BPR
                    last = blk_i == total_blocks - 1
                    if mock_ag:
                        kt_row0 = half * d
                        v_row0 = half * kb
                    else:
                        kt_row0 = (rank * BPR + half) * d
                        v_row0 = rank * r + half * kb

                    if blk_i == 0 and prefetch_blk0:
                        kt_t, v_t = kt0_t, v0_t
                    else:
                        kt_t = []
                        for dc in range(DC):
                            t = pstream.tile([128, kb], MM, name=f"kt{dc}",
                                             tag=f"kt{dc}")
                            nc.sync.dma_start(
                                out=t,
                                in_=kt_src[kt_row0 + dc * 128:
                                           kt_row0 + (dc + 1) * 128, :],
                            )
                            kt_t.append(t)
                        v_t = []
                        v_eng = nc.sync if v_on_sync else nc.gpsimd
                        for kc in range(KC):
                            t = pstream.tile([128, d], MM, name=f"v{kc}",
                                             tag=f"v{kc}")
                            v_eng.dma_start(
                                out=t,
                                in_=v_src[v_row0 + kc * 128:
                                          v_row0 + (kc + 1) * 128, :],
                            )
                            v_t.append(t)

                    # S^T = K_chunk @ Q^T ; P~ = exp(S^T + bias)
                    pt_t = {}
                    for kc in range(KC):
                        for qg in range(NQG):
                            ps = ps_st.tile([128, QG], F32, name="st_ps", tag="st_ps")
                            for dc in range(DC):
                                nc.tensor.matmul(
                                    ps,
                                    kt_t[dc][:, kc * 128:(kc + 1) * 128],
                                    qt_t[dc][:, qg * QG:(qg + 1) * QG],
                                    start=(dc == 0),
                                    stop=(dc == DC - 1),
                                )
                            pt = ppt.tile([128, QG], MM, name="pt", tag=f"pt{kc}_{qg}")
                            nc.scalar.activation(
                                pt, ps, mybir.ActivationFunctionType.Exp,
                                bias=bias_t, scale=1.0,
                            )
                            pt_t[(kc, qg)] = pt

                    rs = ps_rs.tile([128, 2 * NQS], F32, name="rs_ps", tag="rs_ps")
                    if last:
                        # finish all row-sums first so every subtile can be
                        # normalized + stored the moment its PV stops
                        for qs in range(NQS):
                            qg, off = divmod(qs * 128, QG)
                            for kc in range(KC):
                                nc.tensor.matmul(
                                    rs[:, 2 * qs:2 * qs + 2],
                                    pt_t[(kc, qg)][:, off:off + 128],
                                    ones_t,
                                    start=(kc == 0),
                                    stop=(kc == KC - 1),
                                    skip_group_check=True,
                                )
                        nc.vector.tensor_add(oacc_rs, oacc_rs, rs)
                        recip = pout.tile([128, 2 * NQS], F32, name="recip",
                                          tag="recip", bufs=1)
                        nc.vector.reciprocal(recip, oacc_rs)
                        # pre-scale the 15-block accumulator on the (idle)
                        # Activation engine while the PE runs this block's
                        # PV: each output half then needs only ONE fused
                        # DVE op — (pv * recip) + oacc_scaled — before its
                        # store, shortening the drain chain
                        for qs in range(NQS):
                            for nd in range(ND):
                                sl = slice(nd * DW, (nd + 1) * DW)
                                nc.scalar.mul(
                                    oacc[qs][:, sl], oacc[qs][:, sl],
                                    recip[:, 2 * qs:2 * qs + 1])

                    # O += P~^T.T @ V in half-d PSUM groups (nd outermost,
                    # same matmul count/cycles): each 512-wide half retires
                    # as soon as its kc accumulation stops, so the add (and,
                    # in the last block, normalize+store) pipelines behind
                    # the PE at half-subtile granularity — the exposed tail
                    # is only the final half's chain.  Row-sums ride the
                    # nd==0 pass (already done up front in the last block).
                    for qs in range(NQS):
                        qg, off = divmod(qs * 128, QG)
                        lhsTs = [pt_t[(kc, qg)][:, off:off + 128]
                                 for kc in range(KC)]
                        for nd in range(ND):
                            pvh = ps_pv.tile([128, DW], F32, name="pv_ps",
                                             tag="pv_ps")
                            for kc in range(KC):
                                nc.tensor.matmul(
                                    pvh,
                                    lhsTs[kc],
                                    v_t[kc][:, nd * DW:(nd + 1) * DW],
                                    start=(kc == 0),
                                    stop=(kc == KC - 1),
                                    skip_group_check=True,
                                )
                                if nd == 0 and not last:
                                    nc.tensor.matmul(
                                        rs[:, 2 * qs:2 * qs + 2],
                                        lhsTs[kc],
                                        ones_t,
                                        start=(kc == 0),
                                        stop=(kc == KC - 1),
                                        skip_group_check=True,
                                    )
                            sl = slice(nd * DW, (nd + 1) * DW)
                            if last:
                                # single fused op: (pv * recip) + oacc_scaled
                                nc.vector.scalar_tensor_tensor(
                                    oacc[qs][:, sl], pvh,
                                    recip[:, 2 * qs:2 * qs + 1],
                                    oacc[qs][:, sl],
                                    mybir.AluOpType.mult,
                                    mybir.AluOpType.add)
                                # early halves drain on the slow gpsimd
                                # queue; the final ones ride HWDGE
                                idx = 2 * qs + nd
                                if idx < 5:
                                    w_eng = nc.gpsimd
                                else:
                                    w_eng = nc.sync if idx % 2 else nc.scalar
                                w_eng.dma_start(
                                    out=o[qs * 128:(qs + 1) * 128, sl],
                                    in_=oacc[qs][:, sl])
                            elif blk_i == 0:
                                nc.vector.tensor_copy(oacc[qs][:, sl], pvh)
                            else:
                                nc.vector.tensor_add(
                                    oacc[qs][:, sl], oacc[qs][:, sl], pvh)
                    if not last:
                        if blk_i == 0:
                            nc.vector.tensor_copy(oacc_rs, rs)
                        else:
                            nc.vector.tensor_add(oacc_rs, oacc_rs, rs)

    nc.compile()
    return nc


_NC_CACHE = {}


def _get_nc():
    if "fused" not in _NC_CACHE:
        _NC_CACHE["fused"] = build_fused()
    return _NC_CACHE["fused"]


def kernel(inputs, Wq, Wk, Wv):
    inputs = np.ascontiguousarray(inputs, dtype=np.float32)
    XT = np.ascontiguousarray(inputs.T)
    WqT = np.ascontiguousarray(np.asarray(Wq, dtype=np.float32).T)
    WkT = np.ascontiguousarray(np.asarray(Wk, dtype=np.float32).T)
    WvT = np.ascontiguousarray(np.asarray(Wv, dtype=np.float32).T)

    nc = _get_nc()
    R = R_PER_CORE
    in_maps = [
        {
            "xt": np.ascontiguousarray(XT[:, c * R:(c + 1) * R]),
            "wqt": WqT,
            "wkt": WkT,
            "wvt": WvT,
        }
        for c in range(N_CORES)
    ]
    res = run_bass_kernel_spmd(nc, in_maps, core_ids=list(range(N_CORES)))
    out = np.concatenate([res.results[c]["o"] for c in range(N_CORES)], axis=0)
    return out.astype(np.float32)


# revision 47
# speedup vs baseline: 1.0785x; 1.0214x over previous
"""Trainium2 Bass kernel for nn_Attention_Layer (dense transformer attention).

Computes, for X [N, D], Wq/Wk/Wv [D, D]:
    Q = X @ Wq.T ; K = X @ Wk.T ; V = X @ Wv.T
    O = softmax(Q @ K.T, axis=-1) @ V

Strategy (8 NeuronCores, SPMD single launch):
  - Shard rows of X across cores (N=8192 -> 1024 rows/core).
  - Each core computes K_b^T and V_b (each written to an internal DRAM
    bounce and all-gathered immediately, K first), then Q_b^T (kept in SBUF,
    its PE work hiding both collectives' latency).
  - Attention runs in the "transposed" layout: S^T[k, q] tiles are computed
    with K^T chunks stationary and Q^T moving; softmax uses a constant bias
    shift (exact after normalization; no per-row max needed since logits are
    bounded well inside fp32 exp range), so no on-chip transposes and no
    partition-axis reductions are ever needed.  P~ = exp(S^T + bias) chunks
    feed P@V directly as stationary operands; row-sums come from tiny N=2
    matmuls against a ones pair into a shared PSUM bank.  O accumulates in
    SBUF and is normalized once at the end.
  - All matmuls run as float32r (full PE rate at free-dim >= 256).

Overlap engineering (vs the v1 kernel):
  - Projections accumulate with the contraction chunk OUTERMOST across a
    wave of PSUM groups, so the first matmuls issue as soon as the first
    weight/input chunks land instead of after the full 8MB fill.
  - Wq streams into the SBUF buffers vacated by Wk (tag rotation), freeing
    32KB/partition so block 0's K^T/V stream tiles can prefetch during the
    Q projection; the exp activation table is preloaded at kernel start.
  - K^T bounce/gather tensors use a block-contiguous layout ([BPR*d, kb]
    per rank) so every stage-B K^T tile load is a handful of large
    descriptors instead of 128 strided ones; V stream loads ride the
    gpsimd DMA queue to keep the sync sequencer under control.
  - The final key block normalizes and stores each 128-query subtile as
    soon as its PV accumulation stops (row-sums for the block are finished
    first), hiding the normalize/writeout tail under the remaining PV
    matmuls.

AllGather concatenates rank blocks on axis 0; keys are processed in rank-block
order on every core, and the same (rank, local-row) indexing is used for both
K^T and V, so the softmax/PV reduction is consistent (softmax is permutation
invariant over keys).
"""

import numpy as np

import concourse.tile as tile
from concourse import bacc, mybir
from concourse.bass_utils import run_bass_kernel_spmd

N_CORES = 8
N_TOTAL = 8192
D_MODEL = 1024
R_PER_CORE = N_TOTAL // N_CORES  # 1024

F32 = mybir.dt.float32
EXP_BIAS = -45.0  # constant softmax shift; cancels exactly after normalization


def _mm_dt(use_f32r):
    return mybir.dt.float32r if use_f32r else mybir.dt.float32


def build_fused(
    n_cores=N_CORES,
    d=D_MODEL,
    r=R_PER_CORE,
    kb=512,
    exp_bias=EXP_BIAS,
    use_f32r=True,
    mock_ag=False,    # timing/sim builds: skip the collective, read own kvb
    repeat_attn=1,    # timing builds: run stage B this many times
    stream_bufs=2,    # double/triple buffering of the streamed K^T/V tiles
    ps_a_bufs=8,      # stage-A psum wave width (groups in flight)
    pt_bufs=2,        # P~ tile double-buffering across key blocks
    prefetch_blk0=True,  # load block 0's K^T/V during the Q projection
    dc_outer=False,   # contraction-outermost projection waves
    v_on_sync=True,   # stream V tiles on the sync queue (vs gpsimd SWDGE)
):
    """Build the fused QKV + AllGather + attention kernel (SPMD, one program).

    Per-core I/O:
      xt  [d, r]  ExternalInput  — X^T columns for this core's rows
      wqt/wkt/wvt [d, d] ExternalInput — W.T (replicated)
      o   [r, d]  ExternalOutput — this core's output rows
    """
    assert d % 128 == 0 and r % 128 == 0 and kb % 128 == 0
    DC = d // 128            # contraction chunks over d
    NQS = r // 128           # 128-query subtiles per core
    QG = min(512, r)         # query group (free dim) for S^T matmuls
    NQG = r // QG
    KC = kb // 128           # key chunks per key block
    BPR = r // kb            # key blocks per rank block
    DW = min(512, d)         # free-dim slice width over d
    ND = d // DW             # slices of d (for PV matmuls)
    RW = min(512, r)         # free-dim slice width over r
    NR = r // RW

    MM = _mm_dt(use_f32r)  # dtype of all matmul operands (producers round)

    nc = bacc.Bacc("TRN2", target_bir_lowering=False, debug=False, num_devices=n_cores)

    xt = nc.dram_tensor("xt", [d, r], MM, kind="ExternalInput").ap()
    wqt = nc.dram_tensor("wqt", [d, d], MM, kind="ExternalInput").ap()
    wkt = nc.dram_tensor("wkt", [d, d], MM, kind="ExternalInput").ap()
    wvt = nc.dram_tensor("wvt", [d, d], MM, kind="ExternalInput").ap()
    o = nc.dram_tensor("o", [r, d], F32, kind="ExternalOutput").ap()

    # Internal DRAM bounces: K_b^T and V_b, and their all-gathers.  Two
    # separate collectives so attention (which needs K^T + Q^T first) can
    # start while V is still gathering.  ktb is stored block-contiguously:
    # half-block `h` of the rank occupies rows [h*d, (h+1)*d) with kb
    # columns, so stage-B K^T tile loads are large linear descriptors.
    ktb = nc.dram_tensor("ktb", [BPR * d, kb], MM).ap()
    vb = nc.dram_tensor("vb", [r, d], MM).ap()
    ktg = nc.dram_tensor("ktg", [n_cores * BPR * d, kb], MM, addr_space="Shared").ap()
    vg = nc.dram_tensor("vg", [n_cores * r, d], MM, addr_space="Shared").ap()

    with tile.TileContext(nc) as tc:
        with tc.tile_pool(name="persist", bufs=1) as pp:
            # --- persistent tiles ---
            qt_t = []
            for dc in range(DC):
                t = pp.tile([128, r], MM, name=f"qt{dc}", tag=f"qt{dc}")
                qt_t.append(t)
            oacc = []
            for qs in range(NQS):
                t = pp.tile([128, d], F32, name=f"oacc{qs}", tag=f"oacc{qs}")
                oacc.append(t)
            oacc_rs = pp.tile([128, 2 * NQS], F32, name="oacc_rs", tag="oacc_rs")
            # ones pair (fp32r matmuls need even free dims, so the row-sum
            # is computed twice into adjacent psum columns)
            ones_t = pp.tile([128, 2], MM, name="ones_t", tag="ones_t")
            bias_t = pp.tile([128, 1], F32, name="bias_t", tag="bias_t")
            ones_f32 = pp.tile([128, 2], F32, name="ones_f32", tag="ones_f32")
            warm_t = pp.tile([128, 2], F32, name="warm_t", tag="warm_t")
            # block-0 stream prefetch tiles (filled during stage A)
            kt0_t, v0_t = [], []
            if prefetch_blk0:
                for dc in range(DC):
                    kt0_t.append(pp.tile([128, kb], MM, name=f"kt0_{dc}",
                                         tag=f"kt0_{dc}"))
                for kc in range(KC):
                    v0_t.append(pp.tile([128, d], MM, name=f"v0_{kc}",
                                        tag=f"v0_{kc}"))

            # ---------------- Stage A: projections ----------------
            with (
                tc.tile_pool(name="stage_a", bufs=1) as pa,
                tc.tile_pool(name="ps_a", bufs=ps_a_bufs, space="PSUM") as ps_a,
                tc.tile_pool(name="outs_a", bufs=2) as pout_a,
            ):
                # Wk/Wv stay resident; Wq rotates into Wk's buffers (tag
                # reuse with bufs=2) once the K projection retires them.
                # Issue order: (wk, xt) pairs first so the K projection can
                # start as soon as chunk 0 lands; wq goes on the scalar
                # queue so its pool-rotation wait can't stall the sync
                # queue's ktb/vb bounce writes.
                xt_t = []
                wk_t, wv_t, wq_t = [], [], []
                def load_in(t, dram_rows, width, engs, e0=0):
                    n = len(engs)
                    st = width // n
                    for i in range(n):
                        engs[(e0 + i) % n].dma_start(
                            out=t[:, i * st:(i + 1) * st],
                            in_=dram_rows[:, i * st:(i + 1) * st])

                # (wk, xt) fill: all column-first-halves before any second
                # half — K-proj groups whose operands live entirely in
                # columns 0:512 then gate on 4MB instead of the full 8MB
                h2 = d // 2
                for dc in range(DC):
                    t = pa.tile([128, d], MM, name=f"wk{dc}", tag=f"w{dc}", bufs=2)
                    (nc.sync if dc % 2 else nc.gpsimd).dma_start(
                        out=t[:, 0:h2], in_=wkt[dc * 128:(dc + 1) * 128, 0:h2])
                    wk_t.append(t)
                    t = pa.tile([128, r], MM, name=f"xt{dc}", tag=f"xt{dc}")
                    (nc.scalar if dc % 2 else nc.sync).dma_start(
                        out=t[:, 0:h2], in_=xt[dc * 128:(dc + 1) * 128, 0:h2])
                    xt_t.append(t)
                for dc in range(DC):
                    (nc.scalar if dc % 2 else nc.sync).dma_start(
                        out=xt_t[dc][:, h2:d],
                        in_=xt[dc * 128:(dc + 1) * 128, h2:d])
                    (nc.sync if dc % 2 else nc.gpsimd).dma_start(
                        out=wk_t[dc][:, h2:d],
                        in_=wkt[dc * 128:(dc + 1) * 128, h2:d])
                for dc in range(DC):
                    t = pa.tile([128, d], MM, name=f"wv{dc}", tag=f"w{dc}", bufs=2)
                    load_in(t, wvt[dc * 128:(dc + 1) * 128, :], d,
                            [nc.sync, nc.gpsimd], e0=dc)
                    wv_t.append(t)

                # constants + Exp act-table preload: issued after the input
                # loads so their engine/queue time never delays the first
                # DMAs; they only need to be ready for stage B
                nc.vector.memset(bias_t, exp_bias)
                nc.vector.memset(ones_f32, 1.0)
                nc.vector.tensor_copy(ones_t, ones_f32)
                nc.scalar.activation(
                    warm_t, ones_f32, mybir.ActivationFunctionType.Exp,
                    bias=bias_t, scale=1.0,
                )
                # wq rides the gpsimd queue: its loads BLOCK their queue
                # until the K projection retires wk's buffers (in-order
                # FIFO + pool rotation), and gpsimd has nothing behind them
                for dc in range(DC):
                    t = pa.tile([128, d], MM, name=f"wq{dc}", tag=f"w{dc}", bufs=2)
                    load_in(t, wqt[dc * 128:(dc + 1) * 128, :], d,
                            [nc.gpsimd, nc.gpsimd])
                    wq_t.append(t)

                def proj(groups, lhsT_of, rhs_of, sink, outer_waves=0,
                         wave=ps_a_bufs):
                    """out[g] = sum_dc lhsT_of(dc, g).T @ rhs_of(dc, g).

                    The first `outer_waves` waves run with the contraction
                    chunk OUTERMOST across the wave's PSUM groups, so the PE
                    has a full wave of matmuls runnable per arriving input
                    chunk (saturates the DMA-fill window); the rest run
                    group-at-a-time (better p-state behaviour once inputs
                    are resident).
                    """
                    done = 0
                    for _ in range(outer_waves):
                        wv_groups = groups[done:done + wave]
                        done += len(wv_groups)
                        ps_tiles = [
                            ps_a.tile([128, RW], F32, name="ps", tag="ps")
                            for _ in wv_groups
                        ]
                        for dc in range(DC):
                            for t, g in zip(ps_tiles, wv_groups):
                                nc.tensor.matmul(
                                    t, lhsT_of(dc, g), rhs_of(dc, g),
                                    start=(dc == 0), stop=(dc == DC - 1),
                                    skip_group_check=True,
                                )
                        for t, g in zip(ps_tiles, wv_groups):
                            sink(g, t)
                    for g in groups[done:]:
                        ps = ps_a.tile([128, RW], F32, name="ps", tag="ps")
                        for dc in range(DC):
                            nc.tensor.matmul(
                                ps, lhsT_of(dc, g), rhs_of(dc, g),
                                start=(dc == 0), stop=(dc == DC - 1),
                            )
                        sink(g, ps)

                # PSUM->SBUF sink copies alternate DVE / Activation so the
                # copy backlog never stalls PSUM buffer reuse
                _cp = [0]
                def sink_copy(dst, src):
                    if _cp[0] % 2 == 0:
                        nc.vector.tensor_copy(dst, src)
                    else:
                        nc.scalar.copy(dst, src)
                    _cp[0] += 1

                # K_b^T -> ktb (block-contiguous), then gather immediately
                def k_sink(g, ps):
                    oc, rg = g
                    ot = pout_a.tile([128, RW], MM, name="ot", tag="ot")
                    sink_copy(ot, ps)
                    # query... key columns rg*RW.. of K^T live in half-block
                    # rg (RW == kb) at rows oc*128..
                    for h in range(RW // kb):
                        hh = rg * (RW // kb) + h
                        nc.sync.dma_start(
                            out=ktb[hh * d + oc * 128:hh * d + (oc + 1) * 128, :],
                            in_=ot[:, h * kb:(h + 1) * kb],
                        )
                # group order follows the load gates: h0-only groups first
                NO2 = d // 256
                k_groups = (
                    [(oc, 0) for oc in range(NO2)]
                    + [(oc, 1) for oc in range(NO2)]
                    + [(oc, rg) for oc in range(NO2, d // 128)
                       for rg in range(NR)]
                )
                if NR == 1:
                    k_groups = [(oc, 0) for oc in range(d // 128)]
                proj(
                    k_groups,
                    lambda dc, g: wk_t[dc][:, g[0] * 128:(g[0] + 1) * 128],
                    lambda dc, g: xt_t[dc][:, g[1] * RW:(g[1] + 1) * RW],
                    k_sink,
                    outer_waves=1 if dc_outer else 0,
                )
                if not mock_ag:
                    nc.gpsimd.collective_compute(
                        "AllGather",
                        mybir.AluOpType.bypass,
                        ins=[ktb],
                        outs=[ktg],
                        replica_groups=[list(range(n_cores))],
                    )

                # V_b (natural layout) -> vb, then gather immediately so both
                # collectives are in flight while the Q projection runs; PV of
                # the first attention blocks then never waits on the gather.
                def v_sink(g, ps):
                    rc, og = g
                    ot = pout_a.tile([128, DW], MM, name="ot", tag="ot2")
                    sink_copy(ot, ps)
                    nc.sync.dma_start(
                        out=vb[rc * 128:(rc + 1) * 128, og * DW:(og + 1) * DW],
                        in_=ot,
                    )
                proj(
                    [(rc, og) for rc in range(r // 128) for og in range(ND)],
                    lambda dc, g: xt_t[dc][:, g[0] * 128:(g[0] + 1) * 128],
                    lambda dc, g: wv_t[dc][:, g[1] * DW:(g[1] + 1) * DW],
                    v_sink,
                )
                if not mock_ag:
                    nc.gpsimd.collective_compute(
                        "AllGather",
                        mybir.AluOpType.bypass,
                        ins=[vb],
                        outs=[vg],
                        replica_groups=[list(range(n_cores))],
                    )

                # prefetch block 0's K^T/V stream tiles; they only wait on
                # the gathers (or the local bounces under mock_ag), so the
                # DMAs fly while the PE is busy with the Q projection.
                kt_src, v_src = (ktb, vb) if mock_ag else (ktg, vg)
                if prefetch_blk0:
                    for dc in range(DC):
                        nc.sync.dma_start(
                            out=kt0_t[dc],
                            in_=kt_src[dc * 128:(dc + 1) * 128, :],
                        )
                    v_eng = nc.sync if v_on_sync else nc.gpsimd
                    for kc in range(KC):
                        v_eng.dma_start(
                            out=v0_t[kc],
                            in_=v_src[kc * 128:(kc + 1) * 128, :],
                        )

                # Q_b^T stays in SBUF (PE work that overlaps both gathers)
                def q_sink(g, ps):
                    oc, rg = g
                    sink_copy(qt_t[oc][:, rg * RW:(rg + 1) * RW], ps)
                proj(
                    [(oc, rg) for oc in range(d // 128) for rg in range(NR)],
                    lambda dc, g: wq_t[dc][:, g[0] * 128:(g[0] + 1) * 128],
                    lambda dc, g: xt_t[dc][:, g[1] * RW:(g[1] + 1) * RW],
                    q_sink,
                )

            # ---------------- Stage B: attention ----------------
            with (
                tc.tile_pool(name="stream", bufs=stream_bufs) as pstream,
                tc.tile_pool(name="pt_pool", bufs=pt_bufs) as ppt,
                tc.tile_pool(name="ps_st", bufs=3, space="PSUM") as ps_st,
                tc.tile_pool(name="ps_pv", bufs=4, space="PSUM") as ps_pv,
                tc.tile_pool(name="ps_rs", bufs=1, space="PSUM") as ps_rs,
                tc.tile_pool(name="outp", bufs=2) as pout,
            ):
                n_blocks = n_cores * BPR
                total_blocks = repeat_attn * n_blocks
                for blk_i in range(total_blocks):
                    blk = blk_i % n_blocks
                    rank = blk // BPR
                    half = blk % BPR
                    last = blk_i == total_blocks - 1
                    if mock_ag:
                        kt_row0 = half * d
                        v_row0 = half * kb
                    else:
                        kt_row0 = (rank * BPR + half) * d
                        v_row0 = rank * r + half * kb

                    if blk_i == 0 and prefetch_blk0:
                        kt_t, v_t = kt0_t, v0_t
                    else:
                        kt_t = []
                        for dc in range(DC):
                            t = pstream.tile([128, kb], MM, name=f"kt{dc}",
                                             tag=f"kt{dc}")
                            nc.sync.dma_start(
                                out=t,
                                in_=kt_src[kt_row0 + dc * 128:
                                           kt_row0 + (dc + 1) * 128, :],
                            )
                            kt_t.append(t)
                        v_t = []
                        v_eng = nc.sync if v_on_sync else nc.gpsimd
                        for kc in range(KC):
                            t = pstream.tile([128, d], MM, name=f"v{kc}",
                                             tag=f"v{kc}")
                            v_eng.dma_start(
                                out=t,
                                in_=v_src[v_row0 + kc * 128:
                                          v_row0 + (kc + 1) * 128, :],
                            )
                            v_t.append(t)

                    # S^T = K_chunk @ Q^T ; P~ = exp(S^T + bias)
                    pt_t = {}
                    for kc in range(KC):
                        for qg in range(NQG):
                            ps = ps_st.tile([128, QG], F32, name="st_ps", tag="st_ps")
                            for dc in range(DC):
                                nc.tensor.matmul(
                                    ps,
                                    kt_t[dc][:, kc * 128:(kc + 1) * 128],
                                    qt_t[dc][:, qg * QG:(qg + 1) * QG],
                                    start=(dc == 0),
                                    stop=(dc == DC - 1),
                                )
                            pt = ppt.tile([128, QG], MM, name="pt", tag=f"pt{kc}_{qg}")
                            nc.scalar.activation(
                                pt, ps, mybir.ActivationFunctionType.Exp,
                                bias=bias_t, scale=1.0,
                            )
                            pt_t[(kc, qg)] = pt

                    rs = ps_rs.tile([128, 2 * NQS], F32, name="rs_ps", tag="rs_ps")
                    if last:
                        # finish all row-sums first so every subtile can be
                        # normalized + stored the moment its PV stops
                        for qs in range(NQS):
                            qg, off = divmod(qs * 128, QG)
                            for kc in range(KC):
                                nc.tensor.matmul(
                                    rs[:, 2 * qs:2 * qs + 2],
                                    pt_t[(kc, qg)][:, off:off + 128],
                                    ones_t,
                                    start=(kc == 0),
                                    stop=(kc == KC - 1),
                                    skip_group_check=True,
                                )
                        nc.vector.tensor_add(oacc_rs, oacc_rs, rs)
                        recip = pout.tile([128, 2 * NQS], F32, name="recip",
                                          tag="recip", bufs=1)
                        nc.vector.reciprocal(recip, oacc_rs)
                        # pre-scale the 15-block accumulator on the (idle)
                        # Activation engine while the PE runs this block's
                        # PV: each output half then needs only ONE fused
                        # DVE op — (pv * recip) + oacc_scaled — before its
                        # store, shortening the drain chain
                        for qs in range(NQS):
                            for nd in range(ND):
                                sl = slice(nd * DW, (nd + 1) * DW)
                                nc.scalar.mul(
                                    oacc[qs][:, sl], oacc[qs][:, sl],
                                    recip[:, 2 * qs:2 * qs + 1])

                    # O += P~^T.T @ V in half-d PSUM groups (nd outermost,
                    # same matmul count/cycles): each 512-wide half retires
                    # as soon as its kc accumulation stops, so the add (and,
                    # in the last block, normalize+store) pipelines behind
                    # the PE at half-subtile granularity — the exposed tail
                    # is only the final half's chain.  Row-sums ride the
                    # nd==0 pass (already done up front in the last block).
                    for qs in range(NQS):
                        qg, off = divmod(qs * 128, QG)
                        lhsTs = [pt_t[(kc, qg)][:, off:off + 128]
                                 for kc in range(KC)]
                        for nd in range(ND):
                            pvh = ps_pv.tile([128, DW], F32, name="pv_ps",
                                             tag="pv_ps")
                            for kc in range(KC):
                                nc.tensor.matmul(
                                    pvh,
                                    lhsTs[kc],
                                    v_t[kc][:, nd * DW:(nd + 1) * DW],
                                    start=(kc == 0),
                                    stop=(kc == KC - 1),
                                    skip_group_check=True,
                                )
                                if nd == 0 and not last:
                                    nc.tensor.matmul(
                                        rs[:, 2 * qs:2 * qs + 2],
                                        lhsTs[kc],
                                        ones_t,
                                        start=(kc == 0),
                                        stop=(kc == KC - 1),
                                        skip_group_check=True,
                                    )
                            sl = slice(nd * DW, (nd + 1) * DW)
                            if last:
                                # single fused op: (pv * recip) + oacc_scaled
                                nc.vector.scalar_tensor_tensor(
                                    oacc[qs][:, sl], pvh,
                                    recip[:, 2 * qs:2 * qs + 1],
                                    oacc[qs][:, sl],
                                    mybir.AluOpType.mult,
                                    mybir.AluOpType.add)
                                # early halves drain on the slow gpsimd
                                # queue; the final ones ride HWDGE
                                idx = 2 * qs + nd
                                if idx < 5:
                                    w_eng = nc.gpsimd
                                else:
                                    w_eng = nc.sync if idx % 2 else nc.scalar
                                w_eng.dma_start(
                                    out=o[qs * 128:(qs + 1) * 128, sl],
                                    in_=oacc[qs][:, sl])
                            elif blk_i == 0:
                                nc.vector.tensor_copy(oacc[qs][:, sl], pvh)
                            else:
                                nc.vector.tensor_add(
                                    oacc[qs][:, sl], oacc[qs][:, sl], pvh)
                    if not last:
                        if blk_i == 0:
                            nc.vector.tensor_copy(oacc_rs, rs)
                        else:
                            nc.vector.tensor_add(oacc_rs, oacc_rs, rs)

    nc.compile()
    return nc


_NC_CACHE = {}


def _get_nc():
    if "fused" not in _NC_CACHE:
        _NC_CACHE["fused"] = build_fused()
    return _NC_CACHE["fused"]


def kernel(inputs, Wq, Wk, Wv):
    inputs = np.ascontiguousarray(inputs, dtype=np.float32)
    XT = np.ascontiguousarray(inputs.T)
    WqT = np.ascontiguousarray(np.asarray(Wq, dtype=np.float32).T)
    WkT = np.ascontiguousarray(np.asarray(Wk, dtype=np.float32).T)
    WvT = np.ascontiguousarray(np.asarray(Wv, dtype=np.float32).T)

    nc = _get_nc()
    R = R_PER_CORE
    in_maps = [
        {
            "xt": np.ascontiguousarray(XT[:, c * R:(c + 1) * R]),
            "wqt": WqT,
            "wkt": WkT,
            "wvt": WvT,
        }
        for c in range(N_CORES)
    ]
    res = run_bass_kernel_spmd(nc, in_maps, core_ids=list(range(N_CORES)))
    out = np.concatenate([res.results[c]["o"] for c in range(N_CORES)], axis=0)
    return out.astype(np.float32)
